# revision 34
# baseline (speedup 1.0000x reference)
"""DilatedAttention Trainium2 kernel (v3.7).

Structure per 2-tile chunk (g heads, 128 positions/tile):
  - one interleaved DMA loads [q(t0),q(t1) | k.. | v..] (q/k host
    pre-transposed to [(h,d), (q,p')], v standard [pos, (k,d)]);
    consecutive chunk loads alternate sync/scalar issue queues.
  - DVE: one fused prod1_t[(h,d), t, q, k, p'] = Qt*Kt (4 free dims).
  - PE: d-reduction of prod1 via block-ones stationary, 3 heads per
    half-tile into a 3-bank PSUM tile [2, 3, 512] (two tiles ping-pong).
  - ACT: one Exp per half-tile reading its banks in one strided op
    (walk (c, p', k): strided PSUM reads, contiguous SBUF writes),
    writing e_t[h, p', t, q, k].
  - one contiguous SBUF->SBUF DMA (scalar queue) reshapes e_t ->
    e_sb[(h p'), (t q k)].
  - DVE: one fused prod2[pos, t, q, k, d] = e * V.
  - DVE: 3 chunk-fused pairwise tree adds reduce k (the (t,q) dims merge
    into one AP dim, so everything stays <=4 free dims).
  - outputs ev (=sum_k e*V) and raw e; host computes den = sum_k e,
    xn = ev/den (fp64, den rounded to fp32 to match the reference's
    noise profile) and folds 1/(3Z) into the scatter-add.

The whole thing runs as a global depth-2 software pipeline across both
sections (g6 then g5) so the in-order DVE queue never stalls on its own
chunk's PE->ACT->eDMA chain. DVE is the structural bottleneck at ~93%
busy: fp32 tensor_tensor is 1 elem/cycle/partition and the two products
plus the k-tree cannot leave the vector engine (PE only contracts over
partitions, ACT is single-input, GpSimd shares DVE's SBUF port).
fp32 is forced throughout: the reference's normalization sums reach
|Z|=0.0027 while max|out|=392, so xn noise is amplified ~33000x into
the max-rel-err metric; bf16 anywhere fails the 2e-2 gate by 100x.
"""

import os
import sys

sys.path.insert(0, "/opt/trn_rl_repo")

import numpy as np

B, S, H, D = 4, 8192, 16, 64
NG = 3
SEG = [2048, 4096, 8192]
RATE = [1, 2, 4]
GS = [6, 5, 5]
HMIN = [0, 5, 10]
OFF = [0, 1, 2]
P = 128
N6 = 32  # g6 tiles per core
N5 = 24  # g5 tiles per core
SCALE = 1.0 / 8.0

_CACHED_NC = None




def _section_jobs(nc, pools, ind, outd, ps, onesbd, ntiles, g):
    """Return a list of (emit_front, emit_back) closures, one per 2-tile
    chunk, for a global software pipeline across sections."""
    from concourse import mybir

    f32 = mybir.dt.float32
    io, work, small = pools
    gd = g * D
    nch = ntiles // 2
    # head split per half-tile: first half gets 3 heads, second the rest
    h0 = 3 if g >= 3 else g
    h1 = g - h0

    def emit_front(c):
        """Load + prod1 + PE d-reduce + exp + e reshape for chunk c."""
        # flat allocations sized for g=6, viewed per-section (shared tags)
        # qkv-major layout [s, t, gd] so the fused prod1 can merge (t,q)
        qkv_f = io.tile([P, 2 * 3 * 6 * D], f32, tag="qkv")
        qkv = qkv_f[:, 0 : 2 * 3 * gd].rearrange(
            "p (s t x) -> p s t x", s=3, t=2
        )
        # alternate issue queue so consecutive loads ride different rings
        eng = nc.sync if c % 2 == 0 else nc.scalar
        eng.dma_start(out=qkv, in_=ind[c])

        # e in pos-layout [P, (t q k)]
        e_sb_f = small.tile([P, 2 * 36], f32, tag="esb")
        e_sb = e_sb_f[:, 0 : 2 * g * g]
        # e from ACT, layout [h, p', t, q, k] so one contiguous DMA reshapes
        e_t_f = small.tile([2, 64 * 2 * 36], f32, tag="et")
        e_t = e_t_f[:, 0 : 64 * 2 * g * g].rearrange(
            "h (p t q k) -> h p t q k", p=64, t=2, q=g
        )

        # fused prod1_t[(h d), t, q, k, p'] = Qt * Kt (one TT per chunk)
        qb = qkv[:, 0].rearrange("p t (q x) -> p t q x", q=g)
        kb = qkv[:, 1].rearrange("p t (k x) -> p t k x", k=g)
        prod1_f = work.tile([P, 2 * 6 * 6 * D], f32, tag="prod1")
        prod1 = prod1_f[:, 0 : 2 * g * g * D].rearrange(
            "p (t q k x) -> p t q k x", t=2, q=g, k=g
        )
        nc.vector.tensor_mul(
            out=prod1[:],
            in0=qb.unsqueeze(3).broadcast_to([P, 2, g, g, D]),
            in1=kb.unsqueeze(2).broadcast_to([P, 2, g, g, D]),
        )

        for t in range(2):
            # PE d-reduce into PSUM, 2 half-tiles; ACT exp per half
            for hf, (c0, hn) in enumerate([(0, h0), (h0, h1)]):
                psb = ps[hf]
                for i in range(hn):
                    nc.tensor.matmul(
                        psb[:, i, 0:gd],
                        onesbd[:],
                        prod1[:, t, c0 + i, :, :],
                        start=True,
                        stop=True,
                    )
                # Exp over the half's banks: walk (c, p', k) so the SBUF
                # writes stay k-innermost contiguous (PSUM reads pay the
                # stride instead, which is cheap)
                nc.scalar.activation(
                    out=e_t[:, :, t, c0 : c0 + hn, :].rearrange(
                        "h p q k -> h q p k"
                    ),
                    in_=psb[:, 0:hn, 0:gd].rearrange(
                        "h c (k p) -> h c p k", k=g
                    ),
                    func=mybir.ActivationFunctionType.Exp,
                )

        # reshape e_t [h, p', t, q, k] -> e_sb [(h p'), (t q k)]
        # issued from the scalar queue: it already serializes behind the
        # ACTs and keeps the sync queue free for input loads
        nc.scalar.dma_start(
            out=e_sb,
            in_=e_t[:].rearrange("h p t q k -> h (p t q k)"),
        )
        return qkv, e_sb

    def emit_back(c, qkv, e_sb):
        """prod2 + k-tree + store for chunk c (den ships as raw e)."""
        # prod2 chunk [P, t, q, k, d] (contiguous, (t,q) mergeable)
        pev_f = work.tile([P, 2 * 6 * 6 * D], f32, tag="pev")
        pev = pev_f[:, 0 : 2 * gd * g]
        ptq = pev.rearrange("p (tq k x) -> p tq k x", k=g, x=D)  # [(t q),k,d]

        # fused prod2[pos, t, q, k, d] = e[pos, t, q, k] * V[pos, t, k, d]
        vb = qkv[:, 2].rearrange("p t (k x) -> p t k x", k=g)
        eb = e_sb.rearrange("p (t q k) -> p t q k", t=2, q=g)
        nc.vector.tensor_mul(
            out=pev.rearrange("p (t q k x) -> p t q k x", t=2, q=g, k=g),
            in0=eb.unsqueeze(4).broadcast_to([P, 2, g, g, D]),
            in1=vb.unsqueeze(2).broadcast_to([P, 2, g, g, D]),
        )

        # ev out slab [P, (t q d)]
        outb_f = small.tile([P, 2 * 384], f32, tag="outb")
        ev_o = outb_f[:, 0 : 2 * gd].rearrange("p (tq x) -> p tq x", x=D)
        # chunk-fused pairwise k-tree (3 adds), (t,q) merged
        na = g // 2  # 3 for g6, 2 for g5
        hh_f = work.tile([P, 2 * 6 * 3 * D], f32, tag="h1")
        hh = hh_f[:, 0 : 2 * g * na * D].rearrange(
            "p (tq k x) -> p tq k x", k=na, x=D
        )
        nc.vector.tensor_add(
            out=hh[:], in0=ptq[:, :, 0:na, :], in1=ptq[:, :, na : 2 * na, :]
        )
        h2_f = work.tile([P, 2 * 6 * D], f32, tag="h2")
        h2 = h2_f[:, 0 : 2 * g * D].rearrange(
            "p (tq o x) -> p tq o x", o=1, x=D
        )
        nc.vector.tensor_add(
            out=h2, in0=hh[:, :, 0:1, :], in1=hh[:, :, 1:2, :]
        )
        last = hh[:, :, 2:3, :] if g == 6 else ptq[:, :, 4:5, :]
        nc.vector.tensor_add(
            out=ev_o.unsqueeze(2), in0=h2, in1=last
        )

        # store ev + raw e (host derives den = sum_k e in fp64)
        nc.sync.dma_start(
            out=outd[c][:, 0 : 2 * gd], in_=outb_f[:, 0 : 2 * gd]
        )
        nc.sync.dma_start(
            out=outd[c][:, 2 * gd : 2 * gd + 2 * g * g], in_=e_sb
        )

    jobs = []
    for c in range(nch):
        state = {}

        def front(c=c, state=state):
            state["v"] = emit_front(c)

        def back(c=c, state=state):
            emit_back(c, *state["v"])

        jobs.append((front, back))
    return jobs


def _build_nc():
    import concourse.bacc as bacc
    import concourse.tile as tile
    from concourse import mybir

    f32 = mybir.dt.float32
    nc = bacc.Bacc()

    in6 = nc.dram_tensor("in6", [N6 // 2, P, 2 * 3 * 6 * D], f32, kind="ExternalInput")
    in5 = nc.dram_tensor("in5", [N5 // 2, P, 2 * 3 * 5 * D], f32, kind="ExternalInput")
    ob_d = nc.dram_tensor("onesbd", [P, 2], f32, kind="ExternalInput")
    o6 = nc.dram_tensor(
        "o6", [N6 // 2, P, 2 * (6 * D + 36)], f32, kind="ExternalOutput"
    )
    o5 = nc.dram_tensor(
        "o5", [N5 // 2, P, 2 * (5 * D + 25)], f32, kind="ExternalOutput"
    )

    with tile.TileContext(nc) as tc:
        with (
            tc.tile_pool(name="io", bufs=4) as io,
            tc.tile_pool(name="work", bufs=2) as work,
            tc.tile_pool(name="small", bufs=3) as small,
            tc.tile_pool(name="singles", bufs=1) as singles,
            tc.tile_pool(name="psum", bufs=1, space="PSUM") as psum,
        ):
            pools = (io, work, small)
            onesbd = singles.tile([P, 2], f32)
            nc.sync.dma_start(out=onesbd, in_=ob_d[:, :])
            # two 3-bank PSUM tiles, ping-ponged between half-tiles
            ps = [
                psum.tile([2, 3, 512], f32, tag=f"ps{i}", name=f"ps{i}")
                for i in range(2)
            ]

            jobs = _section_jobs(
                nc, pools, in6, o6, ps, onesbd, N6, 6
            ) + _section_jobs(nc, pools, in5, o5, ps, onesbd, N5, 5)
            # global depth-2 software pipeline across both sections
            DEPTH = 2
            for i, (front, _) in enumerate(jobs):
                front()
                if i >= DEPTH:
                    jobs[i - DEPTH][1]()
            for j in jobs[-DEPTH:]:
                j[1]()

    nc.finalize()
    return nc


def _gather(x, b, gi):
    idx = np.arange(OFF[gi], S, RATE[gi])
    return np.ascontiguousarray(x[b, idx, HMIN[gi] : HMIN[gi] + GS[gi], :])


def _transp(a, g):
    """[npos, g, 64] -> transposed tile layout [npos, g*64]: per 128-pos
    tile, row = h*64+d, col = q*64+p'."""
    nt = a.shape[0] // P
    at = a.reshape(nt, 2, 64, g, D).transpose(0, 1, 4, 3, 2)
    return np.ascontiguousarray(at).reshape(nt * P, g * D)


def _interleave(qt, kt, vv, nt, g):
    """rows [nt*P, gd] x3 -> [nt//2, P, 3*2*gd] qkv-major chunk layout."""
    gd = g * D
    q3 = qt.reshape(nt, P, gd)
    k3 = kt.reshape(nt, P, gd)
    v3 = vv.reshape(nt, P, gd)
    a = np.stack([q3, k3, v3], axis=2)  # [nt, P, 3, gd]
    a = a.reshape(nt // 2, 2, P, 3, gd).transpose(0, 2, 3, 1, 4)
    return np.ascontiguousarray(a).reshape(nt // 2, P, 3 * 2 * gd)


def _host_pack(query, key, value):
    in_maps = []
    onesbd = np.zeros((P, 2), dtype=np.float32)
    onesbd[0:64, 0] = 1.0
    onesbd[64:128, 1] = 1.0
    for core in range(8):
        b, role = core // 2, core % 2
        qg0 = _gather(query, b, 0) * SCALE
        kg0 = _gather(key, b, 0)
        vg0 = _gather(value, b, 0)
        qg1 = _gather(query, b, 1) * SCALE
        kg1 = _gather(key, b, 1)
        vg1 = _gather(value, b, 1)
        if role == 0:
            sl6 = slice(0, N6 * P)
            qg2 = _gather(query, b, 2) * SCALE
            kg2 = _gather(key, b, 2)
            vg2 = _gather(value, b, 2)
            q5v = np.concatenate([qg2, qg1[: 8 * P]])
            k5v = np.concatenate([kg2, kg1[: 8 * P]])
            v5v = np.concatenate([vg2, vg1[: 8 * P]])
        else:
            sl6 = slice(N6 * P, 2 * N6 * P)
            q5v = np.ascontiguousarray(qg1[8 * P : 32 * P])
            k5v = np.ascontiguousarray(kg1[8 * P : 32 * P])
            v5v = np.ascontiguousarray(vg1[8 * P : 32 * P])
        in6 = _interleave(
            _transp(qg0[sl6], 6),
            _transp(kg0[sl6], 6),
            vg0[sl6].reshape(N6 * P, 6 * D),
            N6,
            6,
        )
        in5 = _interleave(
            _transp(np.ascontiguousarray(q5v), 5),
            _transp(np.ascontiguousarray(k5v), 5),
            np.ascontiguousarray(v5v).reshape(N5 * P, 5 * D),
            N5,
            5,
        )
        in_maps.append({"in6": in6, "in5": in5, "onesbd": onesbd})
    return in_maps


LAST_EXEC_NS = None


def kernel(query, key, value):
    global _CACHED_NC, LAST_EXEC_NS
    query = np.asarray(query, dtype=np.float32)
    key = np.asarray(key, dtype=np.float32)
    value = np.asarray(value, dtype=np.float32)

    from concourse.bass_utils import run_bass_kernel_spmd

    if _CACHED_NC is None:
        _CACHED_NC = _build_nc()
    nc = _CACHED_NC

    in_maps = _host_pack(query, key, value)
    kw = {}
    if os.environ.get("KERNEL_TRACE"):
        kw = dict(trace=True)
        tdir = os.environ.get("KERNEL_TRACE_DIR")
        if tdir:
            os.makedirs(tdir, exist_ok=True)
            kw["tmpdir"] = tdir
    try:
        res = run_bass_kernel_spmd(nc, in_maps, list(range(8)), **kw)
    except Exception:
        if not kw:
            raise
        kw = {}
        res = run_bass_kernel_spmd(nc, in_maps, list(range(8)))
    if getattr(res, "exec_time_ns", None):
        LAST_EXEC_NS = res.exec_time_ns
    results = res.results

    # ---- host: xn = ev/den (fp64), then fp64 Z, fold 1/(3Z) into scatter ----
    xn6, xn5 = {}, {}
    Z = {}
    for b in range(B):
        for gi in range(NG):
            Z[b, gi] = np.zeros((GS[gi], D), dtype=np.float64)
    for core in range(8):
        b, role = core // 2, core % 2
        r = results[core]
        o6 = np.asarray(r["o6"]).astype(np.float64)
        ev6 = (
            o6[:, :, : 2 * 6 * D]
            .reshape(N6 // 2, P, 2, 6, D)
            .transpose(0, 2, 1, 3, 4)
            .reshape(N6 * P, 6, D)
        )
        den6 = (
            o6[:, :, 2 * 6 * D :]
            .reshape(N6 // 2, P, 2, 6, 6)
            .sum(axis=4, dtype=np.float32)
            .astype(np.float64)
            .transpose(0, 2, 1, 3)
            .reshape(N6 * P, 6)
        )
        o5 = np.asarray(r["o5"]).astype(np.float64)
        ev5 = (
            o5[:, :, : 2 * 5 * D]
            .reshape(N5 // 2, P, 2, 5, D)
            .transpose(0, 2, 1, 3, 4)
            .reshape(N5 * P, 5, D)
        )
        den5 = (
            o5[:, :, 2 * 5 * D :]
            .reshape(N5 // 2, P, 2, 5, 5)
            .sum(axis=4, dtype=np.float32)
            .astype(np.float64)
            .transpose(0, 2, 1, 3)
            .reshape(N5 * P, 5)
        )
        xn6[core] = ev6 / den6[:, :, None]
        xn5[core] = ev5 / den5[:, :, None]
        Z[b, 0] += np.sum(xn6[core], axis=0)
        if role == 0:
            Z[b, 2] += np.sum(xn5[core][: 16 * P], axis=0)
            Z[b, 1] += np.sum(xn5[core][16 * P :], axis=0)
        else:
            Z[b, 1] += np.sum(xn5[core], axis=0)

    out = np.zeros((B, S, H, D), dtype=np.float32)
    for b in range(B):
        rz = [(1.0 / (NG * Z[b, gi])) for gi in range(NG)]
        a_core, b_core = 2 * b, 2 * b + 1
        idx0 = np.arange(OFF[0], S, RATE[0])
        x0 = np.concatenate([xn6[a_core], xn6[b_core]])
        out[b, idx0, HMIN[0] : HMIN[0] + 6, :] += (x0 * rz[0]).astype(np.float32)
        idx2 = np.arange(OFF[2], S, RATE[2])
        out[b, idx2, HMIN[2] : HMIN[2] + 5, :] += (
            xn5[a_core][: 16 * P] * rz[2]
        ).astype(np.float32)
        idx1 = np.arange(OFF[1], S, RATE[1])
        x1 = np.concatenate([xn5[a_core][16 * P :], xn5[b_core]])
        out[b, idx1, HMIN[1] : HMIN[1] + 5, :] += (x1 * rz[1]).astype(np.float32)
    return out


# revision 35
# speedup vs baseline: 1.2130x; 1.2130x over previous
"""DilatedAttention Trainium2 kernel (v3.7).

Structure per 2-tile chunk (g heads, 128 positions/tile):
  - one interleaved DMA loads [q(t0),q(t1) | k.. | v..] (q/k host
    pre-transposed to [(h,d), (q,p')], v standard [pos, (k,d)]);
    consecutive chunk loads alternate sync/scalar issue queues.
  - DVE: one fused prod1_t[(h,d), t, q, k, p'] = Qt*Kt (4 free dims).
  - PE: d-reduction of prod1 via block-ones stationary, 3 heads per
    half-tile into a 3-bank PSUM tile [2, 3, 512] (two tiles ping-pong).
  - ACT: one Exp per half-tile reading its banks in one strided op
    (walk (c, p', k): strided PSUM reads, contiguous SBUF writes),
    writing e_t[h, p', t, q, k].
  - one contiguous SBUF->SBUF DMA (scalar queue) reshapes e_t ->
    e_sb[(h p'), (t q k)].
  - DVE: one fused prod2[pos, t, q, k, d] = e * V.
  - DVE: 3 chunk-fused pairwise tree adds reduce k (the (t,q) dims merge
    into one AP dim, so everything stays <=4 free dims).
  - outputs ev (=sum_k e*V) and raw e; host computes den = sum_k e,
    xn = ev/den (fp64, den rounded to fp32 to match the reference's
    noise profile) and folds 1/(3Z) into the scatter-add.

The whole thing runs as a global depth-2 software pipeline across both
sections (g6 then g5) so the in-order DVE queue never stalls on its own
chunk's PE->ACT->eDMA chain. DVE is the structural bottleneck at ~93%
busy: fp32 tensor_tensor is 1 elem/cycle/partition and the two products
plus the k-tree cannot leave the vector engine (PE only contracts over
partitions, ACT is single-input, GpSimd shares DVE's SBUF port).
fp32 is forced throughout: the reference's normalization sums reach
|Z|=0.0027 while max|out|=392, so xn noise is amplified ~33000x into
the max-rel-err metric; bf16 anywhere fails the 2e-2 gate by 100x.
"""

import os
import sys

sys.path.insert(0, "/opt/trn_rl_repo")

import numpy as np

B, S, H, D = 4, 8192, 16, 64
NG = 3
SEG = [2048, 4096, 8192]
RATE = [1, 2, 4]
GS = [6, 5, 5]
HMIN = [0, 5, 10]
OFF = [0, 1, 2]
P = 128
N6 = 32  # g6 tiles per core
N5 = 24  # g5 tiles per core
SCALE = 1.0 / 8.0

_CACHED_NC = None




def _section_jobs(nc, pools, ind, outd, ps, onesbd, ntiles, g):
    """Return a list of (emit_front, emit_back) closures, one per 2-tile
    chunk, for a global software pipeline across sections."""
    from concourse import mybir

    f32 = mybir.dt.float32
    io, work, small = pools
    gd = g * D
    nch = ntiles // 2
    # head split per half-tile: first half gets 3 heads, second the rest
    h0 = 3 if g >= 3 else g
    h1 = g - h0

    def emit_front(c):
        """Load + prod1 + PE d-reduce + exp + e reshape for chunk c."""
        # flat allocations sized for g=6, viewed per-section (shared tags)
        # qkv-major layout [s, t, gd] so the fused prod1 can merge (t,q)
        qkv_f = io.tile([P, 2 * 3 * 6 * D], f32, tag="qkv")
        qkv = qkv_f[:, 0 : 2 * 3 * gd].rearrange(
            "p (s t x) -> p s t x", s=3, t=2
        )
        nc.sync.dma_start(out=qkv, in_=ind[c])

        # e in pos-layout [P, (t q k)]
        e_sb_f = small.tile([P, 2 * 36], f32, tag="esb")
        e_sb = e_sb_f[:, 0 : 2 * g * g]
        # e from ACT, layout [h, p', t, q, k] so one contiguous DMA reshapes
        e_t_f = small.tile([2, 64 * 2 * 36], f32, tag="et")
        e_t = e_t_f[:, 0 : 64 * 2 * g * g].rearrange(
            "h (p t q k) -> h p t q k", p=64, t=2, q=g
        )

        # fused prod1_t[(h d), t, q, k, p'] = Qt * Kt (one TT per chunk)
        qb = qkv[:, 0].rearrange("p t (q x) -> p t q x", q=g)
        kb = qkv[:, 1].rearrange("p t (k x) -> p t k x", k=g)
        prod1_f = work.tile([P, 2 * 6 * 6 * D], f32, tag="prod1")
        prod1 = prod1_f[:, 0 : 2 * g * g * D].rearrange(
            "p (t q k x) -> p t q k x", t=2, q=g, k=g
        )
        nc.vector.tensor_mul(
            out=prod1[:],
            in0=qb.unsqueeze(3).broadcast_to([P, 2, g, g, D]),
            in1=kb.unsqueeze(2).broadcast_to([P, 2, g, g, D]),
        )

        for t in range(2):
            # PE d-reduce into PSUM, 2 half-tiles; ACT exp per half
            for hf, (c0, hn) in enumerate([(0, h0), (h0, h1)]):
                psb = ps[hf]
                for i in range(hn):
                    nc.tensor.matmul(
                        psb[:, i, 0:gd],
                        onesbd[:],
                        prod1[:, t, c0 + i, :, :],
                        start=True,
                        stop=True,
                    )
                # Exp over the half's banks: walk (c, p', k) so the SBUF
                # writes stay k-innermost contiguous (PSUM reads pay the
                # stride instead, which is cheap)
                nc.scalar.activation(
                    out=e_t[:, :, t, c0 : c0 + hn, :].rearrange(
                        "h p q k -> h q p k"
                    ),
                    in_=psb[:, 0:hn, 0:gd].rearrange(
                        "h c (k p) -> h c p k", k=g
                    ),
                    func=mybir.ActivationFunctionType.Exp,
                )

        # reshape e_t [h, p', t, q, k] -> e_sb [(h p'), (t q k)]
        # issued from the scalar queue: it already serializes behind the
        # ACTs and keeps the sync queue free for input loads
        nc.scalar.dma_start(
            out=e_sb,
            in_=e_t[:].rearrange("h p t q k -> h (p t q k)"),
        )
        return qkv, e_sb

    def emit_back(c, qkv, e_sb):
        """prod2 + k-tree + store for chunk c (den ships as raw e)."""
        # prod2 chunk [P, t, q, k, d] (contiguous, (t,q) mergeable)
        pev_f = work.tile([P, 2 * 6 * 6 * D], f32, tag="pev")
        pev = pev_f[:, 0 : 2 * gd * g]
        ptq = pev.rearrange("p (tq k x) -> p tq k x", k=g, x=D)  # [(t q),k,d]

        # fused prod2[pos, t, q, k, d] = e[pos, t, q, k] * V[pos, t, k, d]
        vb = qkv[:, 2].rearrange("p t (k x) -> p t k x", k=g)
        eb = e_sb.rearrange("p (t q k) -> p t q k", t=2, q=g)
        nc.vector.tensor_mul(
            out=pev.rearrange("p (t q k x) -> p t q k x", t=2, q=g, k=g),
            in0=eb.unsqueeze(4).broadcast_to([P, 2, g, g, D]),
            in1=vb.unsqueeze(2).broadcast_to([P, 2, g, g, D]),
        )

        # ev out slab [P, (t q d)]
        outb_f = small.tile([P, 2 * 384], f32, tag="outb")
        ev_o = outb_f[:, 0 : 2 * gd].rearrange("p (tq x) -> p tq x", x=D)
        # chunk-fused pairwise k-tree (3 adds), (t,q) merged
        na = g // 2  # 3 for g6, 2 for g5
        hh_f = work.tile([P, 2 * 6 * 3 * D], f32, tag="h1")
        hh = hh_f[:, 0 : 2 * g * na * D].rearrange(
            "p (tq k x) -> p tq k x", k=na, x=D
        )
        nc.vector.tensor_add(
            out=hh[:], in0=ptq[:, :, 0:na, :], in1=ptq[:, :, na : 2 * na, :]
        )
        h2_f = work.tile([P, 2 * 6 * D], f32, tag="h2")
        h2 = h2_f[:, 0 : 2 * g * D].rearrange(
            "p (tq o x) -> p tq o x", o=1, x=D
        )
        nc.vector.tensor_add(
            out=h2, in0=hh[:, :, 0:1, :], in1=hh[:, :, 1:2, :]
        )
        last = hh[:, :, 2:3, :] if g == 6 else ptq[:, :, 4:5, :]
        nc.vector.tensor_add(
            out=ev_o.unsqueeze(2), in0=h2, in1=last
        )

        # store ev + raw e (host derives den = sum_k e in fp64)
        nc.sync.dma_start(
            out=outd[c][:, 0 : 2 * gd], in_=outb_f[:, 0 : 2 * gd]
        )
        nc.sync.dma_start(
            out=outd[c][:, 2 * gd : 2 * gd + 2 * g * g], in_=e_sb
        )

    jobs = []
    for c in range(nch):
        state = {}

        def front(c=c, state=state):
            state["v"] = emit_front(c)

        def back(c=c, state=state):
            emit_back(c, *state["v"])

        jobs.append((front, back))
    return jobs


def _build_nc():
    import concourse.bacc as bacc
    import concourse.tile as tile
    from concourse import mybir

    f32 = mybir.dt.float32
    nc = bacc.Bacc()

    in6 = nc.dram_tensor("in6", [N6 // 2, P, 2 * 3 * 6 * D], f32, kind="ExternalInput")
    in5 = nc.dram_tensor("in5", [N5 // 2, P, 2 * 3 * 5 * D], f32, kind="ExternalInput")
    ob_d = nc.dram_tensor("onesbd", [P, 2], f32, kind="ExternalInput")
    o6 = nc.dram_tensor(
        "o6", [N6 // 2, P, 2 * (6 * D + 36)], f32, kind="ExternalOutput"
    )
    o5 = nc.dram_tensor(
        "o5", [N5 // 2, P, 2 * (5 * D + 25)], f32, kind="ExternalOutput"
    )

    with tile.TileContext(nc) as tc:
        with (
            tc.tile_pool(name="io", bufs=4) as io,
            tc.tile_pool(name="work", bufs=2) as work,
            tc.tile_pool(name="small", bufs=3) as small,
            tc.tile_pool(name="singles", bufs=1) as singles,
            tc.tile_pool(name="psum", bufs=1, space="PSUM") as psum,
        ):
            pools = (io, work, small)
            onesbd = singles.tile([P, 2], f32)
            nc.sync.dma_start(out=onesbd, in_=ob_d[:, :])
            # two 3-bank PSUM tiles, ping-ponged between half-tiles
            ps = [
                psum.tile([2, 3, 512], f32, tag=f"ps{i}", name=f"ps{i}")
                for i in range(2)
            ]

            jobs = _section_jobs(
                nc, pools, in6, o6, ps, onesbd, N6, 6
            ) + _section_jobs(nc, pools, in5, o5, ps, onesbd, N5, 5)
            # global depth-2 software pipeline across both sections
            DEPTH = 2
            for i, (front, _) in enumerate(jobs):
                front()
                if i >= DEPTH:
                    jobs[i - DEPTH][1]()
            for j in jobs[-DEPTH:]:
                j[1]()

    nc.finalize()
    return nc


def _gather(x, b, gi):
    idx = np.arange(OFF[gi], S, RATE[gi])
    return np.ascontiguousarray(x[b, idx, HMIN[gi] : HMIN[gi] + GS[gi], :])


def _transp(a, g):
    """[npos, g, 64] -> transposed tile layout [npos, g*64]: per 128-pos
    tile, row = h*64+d, col = q*64+p'."""
    nt = a.shape[0] // P
    at = a.reshape(nt, 2, 64, g, D).transpose(0, 1, 4, 3, 2)
    return np.ascontiguousarray(at).reshape(nt * P, g * D)


def _interleave(qt, kt, vv, nt, g):
    """rows [nt*P, gd] x3 -> [nt//2, P, 3*2*gd] qkv-major chunk layout."""
    gd = g * D
    q3 = qt.reshape(nt, P, gd)
    k3 = kt.reshape(nt, P, gd)
    v3 = vv.reshape(nt, P, gd)
    a = np.stack([q3, k3, v3], axis=2)  # [nt, P, 3, gd]
    a = a.reshape(nt // 2, 2, P, 3, gd).transpose(0, 2, 3, 1, 4)
    return np.ascontiguousarray(a).reshape(nt // 2, P, 3 * 2 * gd)


def _host_pack(query, key, value):
    in_maps = []
    onesbd = np.zeros((P, 2), dtype=np.float32)
    onesbd[0:64, 0] = 1.0
    onesbd[64:128, 1] = 1.0
    for core in range(8):
        b, role = core // 2, core % 2
        qg0 = _gather(query, b, 0) * SCALE
        kg0 = _gather(key, b, 0)
        vg0 = _gather(value, b, 0)
        qg1 = _gather(query, b, 1) * SCALE
        kg1 = _gather(key, b, 1)
        vg1 = _gather(value, b, 1)
        if role == 0:
            sl6 = slice(0, N6 * P)
            qg2 = _gather(query, b, 2) * SCALE
            kg2 = _gather(key, b, 2)
            vg2 = _gather(value, b, 2)
            q5v = np.concatenate([qg2, qg1[: 8 * P]])
            k5v = np.concatenate([kg2, kg1[: 8 * P]])
            v5v = np.concatenate([vg2, vg1[: 8 * P]])
        else:
            sl6 = slice(N6 * P, 2 * N6 * P)
            q5v = np.ascontiguousarray(qg1[8 * P : 32 * P])
            k5v = np.ascontiguousarray(kg1[8 * P : 32 * P])
            v5v = np.ascontiguousarray(vg1[8 * P : 32 * P])
        in6 = _interleave(
            _transp(qg0[sl6], 6),
            _transp(kg0[sl6], 6),
            vg0[sl6].reshape(N6 * P, 6 * D),
            N6,
            6,
        )
        in5 = _interleave(
            _transp(np.ascontiguousarray(q5v), 5),
            _transp(np.ascontiguousarray(k5v), 5),
            np.ascontiguousarray(v5v).reshape(N5 * P, 5 * D),
            N5,
            5,
        )
        in_maps.append({"in6": in6, "in5": in5, "onesbd": onesbd})
    return in_maps


LAST_EXEC_NS = None


def kernel(query, key, value):
    global _CACHED_NC, LAST_EXEC_NS
    query = np.asarray(query, dtype=np.float32)
    key = np.asarray(key, dtype=np.float32)
    value = np.asarray(value, dtype=np.float32)

    from concourse.bass_utils import run_bass_kernel_spmd

    if _CACHED_NC is None:
        _CACHED_NC = _build_nc()
    nc = _CACHED_NC

    in_maps = _host_pack(query, key, value)
    kw = {}
    if os.environ.get("KERNEL_TRACE"):
        kw = dict(trace=True)
        tdir = os.environ.get("KERNEL_TRACE_DIR")
        if tdir:
            os.makedirs(tdir, exist_ok=True)
            kw["tmpdir"] = tdir
    try:
        res = run_bass_kernel_spmd(nc, in_maps, list(range(8)), **kw)
    except Exception:
        if not kw:
            raise
        kw = {}
        res = run_bass_kernel_spmd(nc, in_maps, list(range(8)))
    if getattr(res, "exec_time_ns", None):
        LAST_EXEC_NS = res.exec_time_ns
    results = res.results

    # ---- host: xn = ev/den (fp64), then fp64 Z, fold 1/(3Z) into scatter ----
    xn6, xn5 = {}, {}
    Z = {}
    for b in range(B):
        for gi in range(NG):
            Z[b, gi] = np.zeros((GS[gi], D), dtype=np.float64)
    for core in range(8):
        b, role = core // 2, core % 2
        r = results[core]
        o6 = np.asarray(r["o6"]).astype(np.float64)
        ev6 = (
            o6[:, :, : 2 * 6 * D]
            .reshape(N6 // 2, P, 2, 6, D)
            .transpose(0, 2, 1, 3, 4)
            .reshape(N6 * P, 6, D)
        )
        den6 = (
            o6[:, :, 2 * 6 * D :]
            .reshape(N6 // 2, P, 2, 6, 6)
            .sum(axis=4, dtype=np.float32)
            .astype(np.float64)
            .transpose(0, 2, 1, 3)
            .reshape(N6 * P, 6)
        )
        o5 = np.asarray(r["o5"]).astype(np.float64)
        ev5 = (
            o5[:, :, : 2 * 5 * D]
            .reshape(N5 // 2, P, 2, 5, D)
            .transpose(0, 2, 1, 3, 4)
            .reshape(N5 * P, 5, D)
        )
        den5 = (
            o5[:, :, 2 * 5 * D :]
            .reshape(N5 // 2, P, 2, 5, 5)
            .sum(axis=4, dtype=np.float32)
            .astype(np.float64)
            .transpose(0, 2, 1, 3)
            .reshape(N5 * P, 5)
        )
        xn6[core] = ev6 / den6[:, :, None]
        xn5[core] = ev5 / den5[:, :, None]
        Z[b, 0] += np.sum(xn6[core], axis=0)
        if role == 0:
            Z[b, 2] += np.sum(xn5[core][: 16 * P], axis=0)
            Z[b, 1] += np.sum(xn5[core][16 * P :], axis=0)
        else:
            Z[b, 1] += np.sum(xn5[core], axis=0)

    out = np.zeros((B, S, H, D), dtype=np.float32)
    for b in range(B):
        rz = [(1.0 / (NG * Z[b, gi])) for gi in range(NG)]
        a_core, b_core = 2 * b, 2 * b + 1
        idx0 = np.arange(OFF[0], S, RATE[0])
        x0 = np.concatenate([xn6[a_core], xn6[b_core]])
        out[b, idx0, HMIN[0] : HMIN[0] + 6, :] += (x0 * rz[0]).astype(np.float32)
        idx2 = np.arange(OFF[2], S, RATE[2])
        out[b, idx2, HMIN[2] : HMIN[2] + 5, :] += (
            xn5[a_core][: 16 * P] * rz[2]
        ).astype(np.float32)
        idx1 = np.arange(OFF[1], S, RATE[1])
        x1 = np.concatenate([xn5[a_core][16 * P :], xn5[b_core]])
        out[b, idx1, HMIN[1] : HMIN[1] + 5, :] += (x1 * rz[1]).astype(np.float32)
    return out
